# revision 3
# baseline (speedup 1.0000x reference)
"""Trainium2 Bass kernel for nn_Block_38517266710836.

reference pipeline: channel mixer -> STFT (hann 2048, hop 1024) -> per-frame
recurrence out[f] = (spec[f] + out[f-1]) * transfer -> iSTFT (hann synthesis)
-> overlap-add -> gain -> tanh.

Sharding: 8 cores, data-parallel over (batch, channel-half): core c handles
batch c//2, mixed channels [32*(c%2), +32). Each core receives its batch's
full 64-channel input (the mixer contracts channels) and writes 32 rows.

Pipelined single-pass program per core: mixer chunks, forward-DFT frame
batches, the DVE recurrence scan, and the inverse/overlap-add phase are
interleaved so the scan and evictions hide under PE matmul work.  Forward
evictions and corner-turn copies run on ScalarE (DVE is reserved for the
scan), weights stream on the gpsimd DMA queue, x/y on the sync queue, and
PSUM pools are shared across phases to fit the 8-bank budget.
"""

import numpy as np

WINDOW = 2048
STEP = 1024
CPD = 64
BATCH = 4
TIME = 65536
FRAMES = 64
NJ = 16              # per-frame time chunks (fwd contraction blocks)
NM = 16              # spectral slot chunks
DSH = 32             # mixed channels per core
GCH = TIME // 128    # 512 global 128-sample chunks
GPAD = GCH + 16      # + zero pad (frame 63 reaches t=66560; extra width so
                     # the forward rhs slice [base, base+2048) stays in-bounds)
FC = 4               # frame chunks for the scan layout
FW = 16              # frames per chunk
CB = 17              # chain block: 1 inject/reset col + 16 frame cols
SPECW = NM * DSH * CB  # 8704 free cols per fc block


def _hann(n):
    return (0.5 - 0.5 * np.cos(2.0 * np.pi * np.arange(n) / n)).astype(np.float64)


def _slot_tables():
    """slot s in [0,2048): s<1024 -> Re[k=s]; s==1024 -> Re[1024] (parked in
    Im[0]'s slot, since Im[0] is identically 0); s>1024 -> Im[k=s-1024]."""
    k_of_slot = np.zeros(2048, np.int64)
    is_im = np.zeros(2048, np.bool_)
    for s in range(2048):
        if s < 1024:
            k_of_slot[s] = s
        elif s == 1024:
            k_of_slot[s] = 1024
        else:
            k_of_slot[s] = s - 1024
            is_im[s] = True
    return k_of_slot, is_im


def build_fwd_weights():
    """[2048 n, 2048 slots]: windowed rfft of one frame, slot layout."""
    n = np.arange(WINDOW, dtype=np.float64)
    w = _hann(WINDOW)
    k_of_slot, is_im = _slot_tables()
    ang = 2.0 * np.pi * np.outer(n, k_of_slot.astype(np.float64)) / WINDOW
    W = np.where(is_im[None, :], -np.sin(ang), np.cos(ang))
    W *= w[:, None]
    return W


def build_inv_weights(gain):
    """[2048 slots, 2048 n]: gain * hann * irfft from slot layout."""
    n = np.arange(WINDOW, dtype=np.float64)
    w = _hann(WINDOW)
    k_of_slot, is_im = _slot_tables()
    ang = 2.0 * np.pi * np.outer(k_of_slot.astype(np.float64), n) / WINDOW
    k = k_of_slot
    re_coef = (2.0 - (k == 0) - (k == 1024))[:, None] / WINDOW * np.cos(ang)
    im_coef = -2.0 / WINDOW * np.sin(ang)
    W = np.where(is_im[:, None], im_coef, re_coef)
    W[1024, :] = np.cos(np.pi * n) / WINDOW
    W *= (gain * w)[None, :]
    return W


def build_t_slots(transfer):
    k_of_slot, _ = _slot_tables()
    return np.asarray(transfer, np.float64)[:, k_of_slot]  # [ch, 2048]


def build_pattern(t_slots_core):
    """T-pattern [128, SPECW]: per (m,d) chain block of CB cols:
    col 0 = 0 (reset/inject), cols 1..16 = T[slot(m,kf), d]."""
    pat = np.zeros((128, SPECW), np.float64)
    for m in range(NM):
        for d in range(DSH):
            base = (m * DSH + d) * CB
            pat[:, base + 1: base + CB] = \
                t_slots_core[d, m * 128:(m + 1) * 128][:, None]
    return pat


def emulate(x, transfer, mixer_matrix, gain, wdtype=np.float32):
    """Numpy emulation of the device math (offline validation)."""
    b, c, t = x.shape
    Wf = build_fwd_weights().astype(wdtype).astype(np.float64)
    Wi = build_inv_weights(float(np.asarray(gain).ravel()[0])).astype(wdtype).astype(np.float64)
    Ts = build_t_slots(transfer)
    y = np.einsum('bct,cd->bdt', np.asarray(x, np.float64),
                  np.asarray(mixer_matrix, np.float64))
    yp = np.pad(y, ((0, 0), (0, 0), (0, STEP)))
    out = np.zeros((b, c, t), np.float64)
    for bi in range(b):
        frames = np.stack([yp[bi, :, f * STEP: f * STEP + WINDOW]
                           for f in range(FRAMES)], 1)
        spec = frames.astype(wdtype).astype(np.float64) @ Wf
        st = np.zeros((c, 2048))
        outs = np.zeros_like(spec)
        for f in range(FRAMES):
            st = (spec[:, f].astype(wdtype).astype(np.float64) + st) * Ts
            outs[:, f] = st
        aud = outs.astype(wdtype).astype(np.float64) @ Wi
        acc = np.zeros((c, t + STEP))
        for f in range(FRAMES):
            acc[:, f * STEP: f * STEP + WINDOW] += aud[:, f]
        out[bi] = np.tanh(acc[:, :t])
    return out.astype(np.float32)


# ---------------------------------------------------------------------------
# Device program
# ---------------------------------------------------------------------------

_CACHED_NC = None


def _build_program():
    import concourse.bacc as bacc
    import concourse.mybir as mybir
    from concourse import tile
    from contextlib import ExitStack

    f32 = mybir.dt.float32
    bf16 = mybir.dt.bfloat16
    Alu = mybir.AluOpType

    nc = bacc.Bacc("TRN2", target_bir_lowering=False, debug=False, num_devices=8)
    xb = nc.dram_tensor("xb", [CPD, TIME], bf16, kind="ExternalInput").ap()
    mixw = nc.dram_tensor("mixw", [CPD, DSH], bf16, kind="ExternalInput").ap()
    wf = nc.dram_tensor("wf", [NJ * 128, NM * 128], bf16, kind="ExternalInput").ap()
    wi = nc.dram_tensor("wi", [NM * 128, NJ * 128], bf16, kind="ExternalInput").ap()
    patd = nc.dram_tensor("pat", [128, SPECW], bf16, kind="ExternalInput").ap()
    eyed = nc.dram_tensor("eye", [128, 128], f32, kind="ExternalInput").ap()
    eyebd = nc.dram_tensor("eyeb", [128, 128], bf16, kind="ExternalInput").ap()
    yout = nc.dram_tensor("y", [DSH, TIME], f32, kind="ExternalOutput").ap()

    XCH = 2048           # x streamed in [64, 2048] chunks (16 g-chunks each)
    NXC = TIME // XCH    # 32

    with tile.TileContext(nc) as tc, ExitStack() as ctx:
        persist = ctx.enter_context(tc.tile_pool(name="persist", bufs=1))
        spec = persist.tile([128, FC * SPECW], bf16, tag="spec")
        a_t = persist.tile([128, GPAD * DSH], bf16, tag="a")
        pat = persist.tile([128, SPECW], bf16, tag="pat")
        mx = persist.tile([CPD, DSH], bf16, tag="mx")
        eyeb = persist.tile([128, 128], bf16, tag="eyeb")
        eye = persist.tile([128, 128], f32, tag="eye")

        # PSUM (8 banks of 2KB/partition):
        #   ppA [128,512] f32 x2 = 2 banks  (mixer psum, then phase-I OLA)
        #   ppB [128,512] f32 x2 = 2 banks  (corner-turn psum, then phase-I t4)
        #   sp  [128,1024] f32 x2 = 4 banks (fwd DFT accumulators)
        ppA = ctx.enter_context(tc.tile_pool(name="ppA", bufs=2, space="PSUM"))
        ppB = ctx.enter_context(tc.tile_pool(name="ppB", bufs=2, space="PSUM"))
        sp = ctx.enter_context(tc.tile_pool(name="sp", bufs=2, space="PSUM"))

        xin = ctx.enter_context(tc.tile_pool(name="xin", bufs=2))
        ymp = ctx.enter_context(tc.tile_pool(name="ymp", bufs=2))
        tout = ctx.enter_context(tc.tile_pool(name="tout", bufs=2))
        stg = ctx.enter_context(tc.tile_pool(name="stg", bufs=3))

        # small/early tensors on the sync queue; big weights on gpsimd queue
        nc.sync.dma_start(out=mx[:], in_=mixw[:])
        nc.sync.dma_start(out=eyeb[:], in_=eyebd[:])
        nc.gpsimd.dma_start(out=pat[:], in_=patd[:])
        nc.gpsimd.dma_start(out=eye[:], in_=eyed[:])

        nc.vector.memset(a_t[:, GCH * DSH:], 0.0)
        # chain col 0 of the first fc block must read as 0 (fresh state)
        nc.vector.memset(
            spec[:][:, 0:SPECW].rearrange(
                "p (md c) -> p md c", c=CB)[:, :, 0:1], 0.0)

        def mixer_chunk(xc):
            xt = xin.tile([CPD, XCH], bf16, tag="x", name=f"x{xc}")
            nc.sync.dma_start(out=xt[:], in_=xb[:, xc * XCH:(xc + 1) * XCH])
            pm = ppA.tile([128, 512], f32, tag="pp", name=f"mix{xc}")
            for q in range(4):
                nc.tensor.matmul(
                    pm[q * DSH:(q + 1) * DSH, :],
                    mx[:],
                    xt[:, q * 512:(q + 1) * 512],
                    start=True, stop=True,
                    tile_position=(0, q * DSH))
            ym = ymp.tile([128, 512], bf16, tag="ym", name=f"ym{xc}")
            nc.scalar.copy(ym[:], pm[:])
            # ym[(q,d), tloc]: t = xc*2048 + q*512 + tloc
            for gq in range(4):  # per 4 g-chunks (one psum turn tile)
                pt = ppB.tile([128, 128], bf16, tag="pp", name=f"turn{xc}_{gq}")
                nc.tensor.transpose(
                    pt[:],
                    ym[:, gq * 128: gq * 128 + 128],
                    eyeb[:])
                # pt[tfine, (q2, d)] covers g = xc*16 + q2*4 + gq
                g0 = xc * (XCH // 128)
                dst = a_t[:][:, g0 * DSH:(g0 + 16) * DSH] \
                    .rearrange("p (q2 gq d) -> p q2 gq d", q2=4, gq=4)[
                        :, :, gq, :]
                psrc = pt[:].rearrange("p (q2 d) -> p q2 d", q2=4)
                nc.scalar.copy(dst, psrc)

        def fwd_batch(f16, wf_t):
            # forward DFT: 16 frames batched (N=512); two m per psum tile
            fc = f16
            for qp in range(8):
                ps = sp.tile([128, 1024], f32, tag="sm", name=f"sm{f16}_{qp}")
                for mi in range(2):
                    m = qp * 2 + mi
                    out_ap = ps[:][:, mi * 512:(mi + 1) * 512] \
                        .rearrange("p (d f) -> p f d", f=16)
                    for j in range(NJ):
                        base = (128 * f16 + j) * DSH
                        rhs = a_t[:][:, base: base + 4096] \
                            .rearrange("p (f q) -> p f q", f=16)[:, :, :DSH]
                        nc.tensor.matmul(
                            out_ap,
                            wf_t[:, (j * NM + m) * 128:(j * NM + m + 1) * 128],
                            rhs,
                            start=(j == 0), stop=(j == NJ - 1))
                # single eviction for both m blocks (ScalarE; DVE is scanning)
                src = ps[:].rearrange("p (m2 d f) -> p m2 d f", m2=2, f=16)
                doff = fc * SPECW + (qp * 2) * DSH * CB
                dst = spec[:][:, doff: doff + 2 * DSH * CB] \
                    .rearrange("p (m2 d c) -> p m2 d c", m2=2, c=CB)[:, :, :, 1: 1 + FW]
                nc.scalar.copy(dst, src)

        def scan_block(fc):
            # recurrence scan for frames [16fc, 16fc+16); inject copy to the
            # next block must happen BEFORE the in-place T*u multiply.
            nc.vector.tensor_tensor_scan(
                spec[:, fc * SPECW:(fc + 1) * SPECW],
                pat[:],
                spec[:, fc * SPECW:(fc + 1) * SPECW],
                0.0, Alu.mult, Alu.add)
            if fc + 1 < FC:
                src = spec[:][:, fc * SPECW: (fc + 1) * SPECW] \
                    .rearrange("p (md c) -> p md c", c=CB)[:, :, CB - 1: CB]
                dst = spec[:][:, (fc + 1) * SPECW: (fc + 2) * SPECW] \
                    .rearrange("p (md c) -> p md c", c=CB)[:, :, 0:1]
                nc.vector.tensor_copy(dst, src)
            nc.vector.tensor_mul(
                spec[:, fc * SPECW:(fc + 1) * SPECW],
                spec[:, fc * SPECW:(fc + 1) * SPECW],
                pat[:])

        # ================= phase F (+ scan), pipelined =================
        with ExitStack() as ctxF:
            wp = ctxF.enter_context(tc.tile_pool(name="wfp", bufs=1))
            wf_t = wp.tile([128, NJ * NM * 128], bf16, tag="wf")
            # load m-pair-major so fwd_batch(0)'s first qp groups start early
            for qp in range(8):
                for j in range(NJ):
                    nc.gpsimd.dma_start(
                        out=wf_t[:, (j * NM + qp * 2) * 128:
                                 (j * NM + qp * 2 + 2) * 128],
                        in_=wf[j * 128:(j + 1) * 128,
                               qp * 256:(qp + 1) * 256])

            for xc in range(9):
                mixer_chunk(xc)
            fwd_batch(0, wf_t)
            for xc in range(9, 17):
                mixer_chunk(xc)
            scan_block(0)
            fwd_batch(1, wf_t)
            for xc in range(17, 25):
                mixer_chunk(xc)
            scan_block(1)
            fwd_batch(2, wf_t)
            for xc in range(25, NXC):
                mixer_chunk(xc)
            scan_block(2)
            fwd_batch(3, wf_t)
            scan_block(3)

        # ================= phase I =================
        with ExitStack() as ctxI:
            wp2 = ctxI.enter_context(tc.tile_pool(name="wip", bufs=1))
            wi_t = wp2.tile([128, NM * NJ * 128], bf16, tag="wi")
            for m in range(NM):
                nc.gpsimd.dma_start(
                    out=wi_t[:, m * NJ * 128:(m + 1) * NJ * 128],
                    in_=wi[m * 128:(m + 1) * 128, :])

            yv = yout.rearrange("d (a4 fl t) -> fl d a4 t", fl=4, t=1024)

            for fc in range(FC):
                for rp in range(4):
                    for ji in range(2):
                        j = rp * 2 + ji
                        ps = ppA.tile([128, FW * DSH], f32, tag="pp",
                                      name=f"ola{fc}_{j}")
                        out_full = ps[:].rearrange("p (f d) -> p d f", f=FW)
                        # set A: frames 16fc+fi, chunk j
                        for m in range(NM):
                            base = fc * SPECW + m * DSH * CB
                            rhs = spec[:][:, base: base + DSH * CB] \
                                .rearrange("p (d c) -> p d c", c=CB)[:, :, 1: 1 + FW]
                            nc.tensor.matmul(
                                out_full,
                                wi_t[:, (m * NJ + j) * 128:(m * NJ + j + 1) * 128],
                                rhs, start=(m == 0), stop=False)
                        # set B: frames 16fc+fi-1 (fi>=1), chunk j+8
                        for m in range(NM):
                            base = fc * SPECW + m * DSH * CB
                            rhs = spec[:][:, base: base + DSH * CB] \
                                .rearrange("p (d c) -> p d c", c=CB)[:, :, 1: FW]
                            nc.tensor.matmul(
                                out_full[:, :, 1:],
                                wi_t[:, (m * NJ + j + 8) * 128:(m * NJ + j + 8 + 1) * 128],
                                rhs, start=False,
                                stop=(fc == 0 and m == NM - 1))
                        # boundary: fi=0 gets frame 16fc-1 (chunk j+8)
                        if fc > 0:
                            for m in range(NM):
                                base = (fc - 1) * SPECW + m * DSH * CB + CB - 1
                                rhs = spec[:][:, base: base + DSH * CB] \
                                    .rearrange("p (d c) -> p d c", c=CB)[:, :, 0:1]
                                nc.tensor.matmul(
                                    out_full[:, :, 0:1],
                                    wi_t[:, (m * NJ + j + 8) * 128:(m * NJ + j + 8 + 1) * 128],
                                    rhs, start=False, stop=(m == NM - 1))
                        # tanh eviction
                        tt = tout.tile([128, FW * DSH], f32, tag=f"to{ji}",
                                       name=f"to{fc}_{j}")
                        nc.scalar.activation(
                            tt[:], ps[:], mybir.ActivationFunctionType.Tanh)
                        # corner-turn back + store
                        p4 = ppB.tile([128, 512], f32, tag="pp",
                                      name=f"t4_{fc}_{j}")
                        for r2 in range(4):
                            nc.tensor.transpose(
                                p4[:, r2 * 128:(r2 + 1) * 128],
                                tt[:, r2 * 128:(r2 + 1) * 128],
                                eye[:])
                        st = stg.tile([128, 512], f32, tag="stg",
                                      name=f"stg{fc}_{j}")
                        if ji == 0:
                            nc.vector.tensor_copy(st[:], p4[:])
                        else:
                            nc.scalar.copy(st[:], p4[:])
                        for r2 in range(4):
                            dst = yv[:, :, 4 * fc + r2, j * 128:(j + 1) * 128]
                            nc.sync.dma_start(
                                out=dst,
                                in_=st[:, r2 * 128:(r2 + 1) * 128])
    nc.compile()
    return nc


def _get_nc():
    global _CACHED_NC
    if _CACHED_NC is None:
        _CACHED_NC = _build_program()
    return _CACHED_NC


def kernel(x, transfer, mixer_matrix, gain, _trace=False):
    import ml_dtypes
    from concourse.bass_utils import run_bass_kernel_spmd

    x = np.ascontiguousarray(np.asarray(x, np.float32))
    transfer = np.asarray(transfer, np.float32)
    mixer_matrix = np.asarray(mixer_matrix, np.float32)
    gain = np.asarray(gain, np.float32)

    bf = ml_dtypes.bfloat16
    Wf = build_fwd_weights()
    Wi = build_inv_weights(float(gain.ravel()[0]))
    wf_np = Wf.astype(bf)
    wi_np = Wi.astype(bf)
    Ts = build_t_slots(transfer)
    eye = np.eye(128, dtype=np.float32)
    eyeb = np.eye(128, dtype=np.float64).astype(bf)

    in_maps = []
    for c in range(8):
        b, dh = c // 2, c % 2
        mixw = mixer_matrix[:, dh * DSH:(dh + 1) * DSH].astype(bf)
        patc = build_pattern(Ts[dh * DSH:(dh + 1) * DSH]).astype(bf)
        in_maps.append({
            "xb": x[b].astype(bf),
            "mixw": mixw,
            "wf": wf_np,
            "wi": wi_np,
            "pat": patc,
            "eye": eye,
            "eyeb": eyeb,
        })

    nc = _get_nc()
    res = run_bass_kernel_spmd(nc, in_maps, list(range(8)), trace=_trace)
    out = np.zeros((BATCH, CPD, TIME), np.float32)
    for c in range(8):
        b, dh = c // 2, c % 2
        out[b, dh * DSH:(dh + 1) * DSH] = res.results[c]["y"]
    if _trace:
        return out, res
    return out
